# revision 1
# baseline (speedup 1.0000x reference)
"""Binarized 3x3 conv (sign(x) (*) sign(w)), NCHW 32x128x112x112, OIHW 128x128x3x3,
stride 1, pad 1 -> out 32x128x112x112 f32.

Strategy: data-parallel over batch N across 8 NeuronCores (4 images/core,
weights replicated). Per core: binarize x on ScalarE into a zero-padded
SBUF image [C=128, 114, WP], then conv = accumulating 128x128 matmuls per
output tile (contraction over C on the partition dim), PSUM f32 accumulate
(exact: sums of +-1 are small integers), drain to SBUF, DMA out. Weights
are binarized + transposed to [C, pos, O] on-chip via the PE transpose path.

Modes (BCONV_MODE env; default fp8dr2b is the tuned one):
  bf16    — 9 normal matmuls per 4-row output tile (FD=448).        ~219 us
  fp8dr   — fp8; kh=0/1 pairs as DoubleRow matmuls (flat WP=128
            row-padded layout), kh=2 as 3 normal matmuls.           ~170 us
  fp8dr2  — + (kh2,kw0)+(kh2,kw1) paired via a col-shifted second
            plane (copy on GpSimd — too slow, kept for reference).  ~228 us
  fp8dr2b — 4 DoubleRow + 1 normal matmul per tile; shifted plane
            built by alternating ACT re-sign / DVE copy; software-
            pipelined binarize prefetch; 16-row store groups.       ~154 us
"""

import os
from contextlib import ExitStack

import numpy as np

import concourse.bass as bass
import concourse.tile as tile
import concourse.mybir as mybir
from concourse import bacc, masks
from concourse.bass_utils import run_bass_kernel_spmd
from concourse.tile_rust import add_dep_helper

F32 = mybir.dt.float32
BF16 = mybir.dt.bfloat16
FP8 = mybir.dt.float8e4

MODE = os.environ.get("BCONV_MODE", "fp8dr2b")

N, C, H, W, O = 32, 128, 112, 112, 128
KH = KW = 3
NCORES = 8
NPC = N // NCORES  # images per core
HP = H + 2  # padded rows (row 0 and 113 are zero pad)
RCHUNK = 16  # input rows per load chunk
NCHUNK = H // RCHUNK  # 7
R = 4  # output rows per psum tile
TILES = H // R  # 28
GROUP = 4  # tiles per output store chunk (< psum bufs: no intra-group choke)
NGROUP = TILES // GROUP  # 7
GR = GROUP * R  # 16 rows per store

_built = {}


def _build(mode):
    fp8 = mode in ("fp8dr", "fp8dr2", "fp8dr2b")
    two_plane = mode in ("fp8dr2", "fp8dr2b")
    strip_split = mode == "fp8dr2b"
    XDT = FP8 if fp8 else BF16
    WP = 128 if fp8 else H + 2  # row stride; fp8 flat trick needs %16 == 0

    nc = bacc.Bacc(
        "TRN2", target_bir_lowering=False, debug=False, num_devices=NCORES
    )
    x_ext = nc.dram_tensor("x", [NPC, C, H, W], F32, kind="ExternalInput")
    w_ext = nc.dram_tensor("weights", [O, C, KH, KW], F32, kind="ExternalInput")
    out_ext = nc.dram_tensor("out", [NPC, O, H, W], F32, kind="ExternalOutput")

    with tile.TileContext(nc) as tc, ExitStack() as ctx:
        wpool = ctx.enter_context(tc.tile_pool(name="wpool", bufs=1))
        psum = ctx.enter_context(tc.tile_pool(name="psum", bufs=1, space="PSUM"))
        deep = mode == "fp8dr2b"
        inpool = ctx.enter_context(tc.tile_pool(name="inpool", bufs=6 if deep else 3))
        xpool = ctx.enter_context(tc.tile_pool(name="xpool", bufs=3 if deep else 2))
        stpool = ctx.enter_context(tc.tile_pool(name="stpool", bufs=3))

        # ---- weights: load f32 [O, I*9], binarize, PE-transpose to [C, pos, O]
        w_sb = wpool.tile([O, C * KH * KW], F32)
        # first DMA issued: everything at the head chains on sign(w)
        w_dma = nc.sync.dma_start(
            out=w_sb[:], in_=w_ext.rearrange("o i kh kw -> o (i kh kw)")
        )
        wsign = wpool.tile([O, C * KH * KW], BF16)
        nc.scalar.sign(wsign[:], w_sb[:])
        ident = wpool.tile([128, 128], BF16)
        masks.make_identity(nc, ident[:])
        wT = wpool.tile([C, KH * KW, O], XDT)
        wsv = wsign.rearrange("o (i p) -> o p i", p=KH * KW)
        for p in range(KH * KW):
            tps = psum.tile([128, 128], BF16, name="tps", tag="tps", bufs=2)
            nc.tensor.transpose(out=tps[:], in_=wsv[:, p, :], identity=ident[:])
            nc.vector.tensor_copy(wT[:, p, :], tps[:])

        store_eng = nc.scalar if (two_plane and not strip_split) else nc.gpsimd

        xps = {}

        def emit_binarize(n):
            xps[n] = _emit_binarize_body(n)

        def _emit_binarize_body(n):
            if two_plane:
                # plane 0: padded sign image; plane 1: same, shifted left 1 col
                # (lets the (kh=2,kw=0)+(kh=2,kw=1) pair run as DoubleRow with
                # pair stride = plane stride). Junk columns >= 114 (plane 0)
                # / >= 112 (plane 1) only ever land in dropped output columns,
                # so they are left uninitialized.
                xp = xpool.tile([C, 2, HP, WP], XDT, name="xp")
                nc.vector.memset(xp[:, :, 0, 0 : W + 2], 0.0)
                nc.vector.memset(xp[:, :, HP - 1, 0 : W + 2], 0.0)
                nc.vector.memset(xp[:, 0, 1 : H + 1, 0], 0.0)
                nc.vector.memset(xp[:, 0, 1 : H + 1, W + 1], 0.0)
            else:
                xp = xpool.tile([C, 1, HP, WP], XDT, name="xp")
                nc.vector.memset(xp[:, 0, 0, 0 : W + 2], 0.0)
                nc.vector.memset(xp[:, 0, HP - 1, 0 : W + 2], 0.0)
                nc.vector.memset(xp[:, 0, 1 : H + 1, 0], 0.0)
                nc.vector.memset(xp[:, 0, 1 : H + 1, W + 1], 0.0)
                if fp8:
                    # junk columns do enter DR rhs flat windows; keep finite
                    nc.vector.memset(xp[:, 0, :, W + 2 : WP], 0.0)
            if strip_split and n == 0:
                # fast start: a small first chunk so the first sign (and the
                # first conv matmuls behind it) clear ScalarE's serial head
                # chain ~3us earlier
                bounds = [(0, 8)] + [(8 + 16 * i, 24 + 16 * i) for i in range(6)]
                bounds.append((104, H))
            else:
                bounds = [(k * RCHUNK, (k + 1) * RCHUNK) for k in range(NCHUNK)]
            for k, (a, b) in enumerate(bounds):
                xin = inpool.tile([C, RCHUNK, W], F32, name="xin")
                ld = nc.sync.dma_start(out=xin[: C, : b - a], in_=x_ext[n, :, a:b, :])
                if strip_split and n == 0 and k < 3:
                    # don't let the first x loads share SDMA round-robin with
                    # the weight load; the whole head chains on sign(w)
                    add_dep_helper(ld.ins, w_dma.ins, reason="w load first")
                rows = slice(1 + a, 1 + b)
                nc.scalar.sign(xp[:, 0, rows, 1 : 1 + W], xin[: C, : b - a])
                if two_plane and strip_split:
                    # plane1[h, w] = sign(x[h-1, w]): same chunk, shifted col.
                    # Alternate ACT re-sign / DVE copy to spread the load.
                    if k % 2 == 0:
                        nc.vector.tensor_copy(
                            xp[:, 1, rows, 0:W], xp[:, 0, rows, 1 : 1 + W]
                        )
                    else:
                        nc.scalar.sign(xp[:, 1, rows, 0:W], xin[: C, : b - a])
                elif two_plane:
                    nc.gpsimd.tensor_copy(
                        xp[:, 1, rows, 0:W], xp[:, 0, rows, 1 : 1 + W]
                    )
            return xp

        def emit_compute(n):
            xp = xps.pop(n)
            xp_pstride = xp.ap[0][0]
            plane_stride = xp.ap[1][0]
            for g in range(NGROUP):
                stage = stpool.tile([O, GR, W], F32, name="stage")
                for tt in range(GROUP):
                    t = g * GROUP + tt
                    r0 = t * R
                    if fp8:
                        ps = psum.tile([O, R * WP], F32, name="ps", tag="mm", bufs=6)
                        psv = ps.rearrange("o (r w) -> o r w", w=WP)
                        for kw in range(KW):
                            # kh=0/kh=1 pair as one DoubleRow matmul over the
                            # flat row-padded layout (overlapping windows)
                            rhs = bass.AP(
                                tensor=xp.tensor,
                                offset=xp.offset + r0 * WP + kw,
                                ap=[[xp_pstride, C], [WP, 2], [1, R * WP]],
                            )
                            nc.tensor.matmul(
                                out=ps[:],
                                lhsT=wT[:, kw : kw + 2 * KW : KW, :],
                                rhs=rhs,
                                perf_mode=mybir.MatmulPerfMode.DoubleRow,
                                start=(kw == 0),
                                stop=False,
                            )
                        if two_plane:
                            # (kh=2, kw=0) + (kh=2, kw=1) via the shifted plane
                            rhs = bass.AP(
                                tensor=xp.tensor,
                                offset=xp.offset + (r0 + 2) * WP,
                                ap=[[xp_pstride, C], [plane_stride, 2], [1, R * WP]],
                            )
                            nc.tensor.matmul(
                                out=ps[:],
                                lhsT=wT[:, 2 * KW : 2 * KW + 2, :],
                                rhs=rhs,
                                perf_mode=mybir.MatmulPerfMode.DoubleRow,
                                start=False,
                                stop=False,
                            )
                            nc.tensor.matmul(
                                out=psv[:, :, :W],
                                lhsT=wT[:, 2 * KW + 2, :],
                                rhs=xp[:, 0, r0 + 2 : r0 + 2 + R, 2 : 2 + W],
                                start=False,
                                stop=True,
                            )
                        else:
                            for kw in range(KW):
                                nc.tensor.matmul(
                                    out=psv[:, :, :W],
                                    lhsT=wT[:, 2 * KW + kw, :],
                                    rhs=xp[:, 0, r0 + 2 : r0 + 2 + R, kw : kw + W],
                                    start=False,
                                    stop=(kw == KW - 1),
                                )
                        drain_src = psv[:, :, :W]
                    else:
                        ps = psum.tile([O, R, W], F32, name="ps", tag="mm", bufs=6)
                        for kh in range(KH):
                            for kw in range(KW):
                                p = kh * KW + kw
                                nc.tensor.matmul(
                                    out=ps[:],
                                    lhsT=wT[:, p, :],
                                    rhs=xp[:, 0, r0 + kh : r0 + kh + R, kw : kw + W],
                                    start=(p == 0),
                                    stop=(p == KH * KW - 1),
                                )
                        drain_src = ps[:]
                    nc.vector.tensor_copy(stage[:, tt * R : (tt + 1) * R, :], drain_src)
                last = n == NPC - 1 and g == NGROUP - 1
                if last:
                    # split the final store so the tail DMA is short
                    for a, b in ((0, GR // 2), (GR // 2, GR)):
                        store_eng.dma_start(
                            out=out_ext[n, :, g * GR + a : g * GR + b, :],
                            in_=stage[:, a:b, :],
                        )
                else:
                    store_eng.dma_start(
                        out=out_ext[n, :, g * GR : (g + 1) * GR, :], in_=stage[:]
                    )

        if strip_split:
            # software pipeline: binarize(n+2) is emitted before compute(n+1)
            # so the next image's DVE/ACT prep never queues behind the
            # current image's PSUM drains (engine FIFOs = program order)
            emit_binarize(0)
            emit_binarize(1)
            for n in range(NPC):
                emit_compute(n)
                if n + 2 < NPC:
                    emit_binarize(n + 2)
        else:
            for n in range(NPC):
                emit_binarize(n)
                emit_compute(n)
    nc.compile()
    return nc


def run(x, weights, mode=MODE, **spmd_kwargs):
    """Run on 8 cores; returns (full output [32,128,112,112], BassKernelResults)."""
    x = np.ascontiguousarray(np.asarray(x, dtype=np.float32))
    weights = np.ascontiguousarray(np.asarray(weights, dtype=np.float32))
    assert x.shape == (N, C, H, W) and weights.shape == (O, C, KH, KW)
    if mode not in _built:
        _built[mode] = _build(mode)
    nc = _built[mode]
    core_ids = list(range(NCORES))
    in_maps = [
        {"x": x[i * NPC : (i + 1) * NPC], "weights": weights} for i in range(NCORES)
    ]
    res = run_bass_kernel_spmd(nc, in_maps, core_ids, **spmd_kwargs)
    out = np.concatenate([res.results[i]["out"] for i in range(NCORES)], axis=0)
    return out, res


def kernel(x, weights):
    out, _ = run(x, weights)
    return out

